# revision 16
# baseline (speedup 1.0000x reference)
"""AttentionLSTM Trainium2 kernel, 8-core SPMD, tensor-parallel over 4H.

Core k owns H-slice [128k, 128(k+1)) of each gate (512 act columns
[i_k | f_k | o_k | g_k]) plus the matching c/h state slice.  Per step one
AllGather carries [partial_scores(16 f32) | hT-slice(128)] per batch row
(bf16): gathered hT blocks are the lhsT k-tiles for h@Wh; the 8 partial
score blocks sum to the full attention scores on every core.  The
attention contribution is sum_p w[n,p] * B[n,p,j] with
B[n,p,j] = sum_h Af[n,h,p] Wattn[h,j] precomputed (prologue), applied as
16 diag(w_p) @ B_p matmuls into the same PSUM bank as x@Wx (prefetched
three steps deep so the PE stays busy through the AllGather window) and
h@Wh.

Gate math runs sigmoid-free on a doubled cell state C=2c:
  th = tanh(act/2) (g columns pre-doubled host-side so th_g = tanh(g)),
  sf = 0.5*th_f+0.5, C' = sf*C + (th_i+1)*th_g,  tanh(c') = Tanh(C'*0.5),
  h = (0.5*th_o+0.5) * tanh(c').
h ships as transpose(h) via one PE transpose; partial scores are
sum_hd afn[n,p,hd]*h[n,hd] via mul/fold/reduce on DVE (diag build is
split across DVE and GpSimd).
"""
import numpy as np

from concourse import bacc, tile
from concourse import mybir
from concourse.bass_utils import run_bass_kernel_spmd
from concourse.tile_rust import add_dep_helper

N, T, D, H = 128, 64, 1024, 1024
P16 = 16
NC = 8
HS = H // NC      # 128
JS = 4 * HS       # 512
KT = D // 128     # 8
PW = 2 * P16 + 128  # 160 payload cols: 32 bf16 (16 f32 scores) + 128 hT

F32 = mybir.dt.float32
BF16 = mybir.dt.bfloat16
BF16_NP = mybir.dt.np(mybir.dt.bfloat16)
RG = [list(range(NC))]
PF = 3  # x@Wx prefetch depth

_nc_cache = None


def _build():
    nc = bacc.Bacc("TRN2", target_bir_lowering=False, debug=False, num_devices=NC)

    xT_d = nc.dram_tensor("xT", [T, 128, KT, N], BF16, kind="ExternalInput").ap()
    wx_d = nc.dram_tensor("wx", [128, KT, JS], BF16, kind="ExternalInput").ap()
    wh_d = nc.dram_tensor("wh", [128, KT, JS], BF16, kind="ExternalInput").ap()
    wattn_d = nc.dram_tensor("wattn", [128, KT, JS], BF16, kind="ExternalInput").ap()
    b_d = nc.dram_tensor("bvec", [1, JS], BF16, kind="ExternalInput").ap()
    ident_d = nc.dram_tensor("ident", [128, 128], BF16, kind="ExternalInput").ap()
    afn_d = nc.dram_tensor("afn", [128, P16, HS], BF16, kind="ExternalInput").ap()
    afT_d = nc.dram_tensor("afT", [128, KT, P16, N], BF16, kind="ExternalInput").ap()
    h0_d = nc.dram_tensor("h0", [128, HS], F32, kind="ExternalInput").ap()
    out_d = nc.dram_tensor("out", [T, HS, N], BF16, kind="ExternalOutput").ap()

    with tile.TileContext(nc) as tc:
        with (
            tc.tile_pool(name="const", bufs=1) as cp,
            tc.tile_pool(name="state", bufs=2) as sp,
            tc.tile_pool(name="work", bufs=2) as wp,
            tc.tile_pool(name="xpool", bufs=PF + 1) as xp,
            tc.tile_pool(name="paypool", bufs=2) as yp,
            tc.tile_pool(name="psum", bufs=PF + 1, space="PSUM") as pp,
            tc.tile_pool(name="tpsum", bufs=2, space="PSUM") as tp,
            tc.tile_pool(name="dram", bufs=2, space="DRAM") as dp,
        ):
            # ---------------- constants ----------------
            wx_b = cp.tile([128, KT, JS], BF16, name="wx_b")
            nc.sync.dma_start(out=wx_b[:], in_=wx_d[:])
            wh_b = cp.tile([128, KT, JS], BF16, name="wh_b")
            nc.sync.dma_start(out=wh_b[:], in_=wh_d[:])
            b_b = cp.tile([1, JS], BF16, name="b_b")
            nc.sync.dma_start(out=b_b[:], in_=b_d[:])
            ident_b = cp.tile([128, 128], BF16, name="ident_b")
            nc.sync.dma_start(out=ident_b[:], in_=ident_d[:])
            afn_b = cp.tile([128, P16, HS], BF16, name="afn_b")
            nc.sync.dma_start(out=afn_b[:], in_=afn_d[:])
            ones_b = cp.tile([1, 128], BF16, name="ones_b")
            nc.vector.memset(ones_b[:], 1.0)
            ident_rep = cp.tile([128, P16, 128], BF16, name="ident_rep")
            nc.vector.tensor_copy(
                ident_rep[:], ident_b[:].unsqueeze(1).broadcast_to([128, P16, 128])
            )
            B_s = cp.tile([128, P16, JS], BF16, name="B_s")

            # ---------------- prologue ----------------
            c2_cur = sp.tile([128, HS], F32, name="c2", tag="c2")
            pay = yp.tile([128, PW], BF16, name="pay", tag="pay")
            with (
                tc.tile_pool(name="prol", bufs=1) as pr,
                tc.tile_pool(name="prolp", bufs=2, space="PSUM") as prp,
            ):
                wat_b = pr.tile([128, KT, JS], BF16, name="wat_b")
                nc.sync.dma_start(out=wat_b[:], in_=wattn_d[:])
                afT_b = pr.tile([128, KT, P16, N], BF16, name="afT_b")
                nc.sync.dma_start(out=afT_b[:], in_=afT_d[:])
                h0_t = pr.tile([128, HS], F32, name="h0_t")
                nc.sync.dma_start(out=h0_t[:], in_=h0_d[:])
                nc.vector.tensor_scalar(
                    out=c2_cur[:], in0=h0_t[:], scalar1=2.0, scalar2=None,
                    op0=mybir.AluOpType.mult,
                )
                h0_b = pr.tile([128, HS], BF16, name="h0_b")
                nc.vector.tensor_copy(h0_b[:], h0_t[:])

                # B[n,p,j] = sum_h Af[n,h,p] Wattn[h,j] + b[j]
                # (softmax weights sum to 1, so folding the bias row into
                #  every B_p replaces the per-step bias matmul)
                for p in range(P16):
                    bp = prp.tile([128, JS], F32, name="bp", tag="bp")
                    for kk in range(KT):
                        nc.tensor.matmul(
                            out=bp[:],
                            lhsT=afT_b[:, kk, p, :],
                            rhs=wat_b[:, kk, :],
                            start=(kk == 0), stop=False,
                        )
                    nc.tensor.matmul(
                        out=bp[:], lhsT=ones_b[:], rhs=b_b[:],
                        start=False, stop=True,
                    )
                    nc.scalar.copy(out=B_s[:, p, :], in_=bp[:])

                # payload 0 from h0: [pscores | h0T]
                hTp = tp.tile([128, 128], BF16, name="hTp", tag="tp")
                nc.tensor.transpose(hTp[:], h0_b[:], ident_b[:])
                nc.scalar.copy(out=pay[:, 2 * P16 :], in_=hTp[:])
                prod0 = pr.tile([128, P16, HS], BF16, name="prod0")
                nc.vector.tensor_mul(
                    prod0[:], afn_b[:],
                    h0_b[:].unsqueeze(1).broadcast_to([128, P16, HS]),
                )
                pf0 = pr.tile([128, P16, HS // 2], BF16, name="pf0")
                nc.vector.tensor_add(
                    pf0[:], prod0[:, :, 0 : HS // 2], prod0[:, :, HS // 2 :]
                )
                nc.vector.tensor_reduce(
                    pay[:, 0 : 2 * P16].bitcast(F32), pf0[:],
                    mybir.AxisListType.X, mybir.AluOpType.add,
                )
            bin_ = dp.tile([128, PW], BF16, name="bin", tag="bin")
            nc.sync.dma_start(out=bin_[:], in_=pay[:])

            # ---------------- recurrence ----------------
            def xwx_prefetch(t):
                """x_t @ Wx + b into a fresh act psum bank."""
                xt = xp.tile([128, KT, N], BF16, name="xt", tag="xt")
                nc.sync.dma_start(out=xt[:], in_=xT_d[t])
                act = pp.tile([128, JS], F32, name="act", tag="act")
                mms = []
                for kk in range(KT):
                    mms.append(nc.tensor.matmul(
                        out=act[:],
                        lhsT=xt[:, kk, :],
                        rhs=wx_b[:, kk, :],
                        start=(kk == 0), stop=False,
                    ))
                return act, mms

            acts = [xwx_prefetch(t) for t in range(PF - 1)]
            prev_pe_last = None   # last PE instr queued (order pin)

            for t in range(T):
                # ---- gather payload of h_t (bin_ written last iteration) ----
                bout = dp.tile([NC * 128, PW], BF16, addr_space="Shared",
                               name="bout", tag="bout")
                ag = nc.gpsimd.collective_compute(
                    "AllGather", mybir.AluOpType.bypass, replica_groups=RG,
                    ins=[bin_[:].opt()], outs=[bout[:].opt()],
                )
                # x@Wx prefetch queued to run right at AG completion: its
                # cold matmuls fill the ~4us gather-DMA window and re-warm
                # HAM so the Wh/diag matmuls run at full clock.
                if t + PF - 1 < T:
                    a, mms = xwx_prefetch(t + PF - 1)
                    acts.append((a, mms))
                    add_dep_helper(mms[0].ins, ag.ins, sync=True,
                                   reason="xwx-at-ag-completion")
                    if prev_pe_last is not None:
                        add_dep_helper(mms[0].ins, prev_pe_last.ins,
                                       sync=False, reason="pe-order")
                    prev_pe_last = mms[-1]
                gsrc = bout[:].rearrange("(kk p) f -> p kk f", kk=NC)
                # scores chunk first (small, scalar HWDGE), hT split sync/scalar
                gs = wp.tile([128, NC, 2 * P16], BF16, name="gs", tag="gs")
                nc.scalar.dma_start(out=gs[:], in_=gsrc[:, :, 0 : 2 * P16])
                g_lo = wp.tile([128, 4, 128], BF16, name="g_lo", tag="g_lo")
                nc.sync.dma_start(out=g_lo[:], in_=gsrc[:, 0:4, 2 * P16 :])
                g_hi = wp.tile([128, 4, 128], BF16, name="g_hi", tag="g_hi")
                nc.scalar.dma_start(out=g_hi[:], in_=gsrc[:, 4:8, 2 * P16 :])

                # ---- softmax weights -> diag halves ----
                scores = wp.tile([128, P16], F32, name="scores", tag="scores")
                nc.vector.tensor_reduce(
                    scores[:],
                    gs[:].bitcast(F32).rearrange("n kk q -> n q kk"),
                    mybir.AxisListType.X, mybir.AluOpType.add,
                )
                e_b = wp.tile([128, P16], BF16, name="e_b", tag="e_b")
                den = wp.tile([128, 1], F32, name="den", tag="den")
                nc.scalar.activation(
                    out=e_b[:], in_=scores[:], func=mybir.ActivationFunctionType.Exp,
                    scale=1.0 / 32.0, accum_out=den[:],
                )
                rden = wp.tile([128, 1], F32, name="rden", tag="rden")
                nc.vector.reciprocal(rden[:], den[:])
                e2 = wp.tile([128, P16], BF16, name="e2", tag="e2")
                nc.vector.tensor_scalar(
                    out=e2[:], in0=e_b[:], scalar1=rden[:], scalar2=None,
                    op0=mybir.AluOpType.mult,
                )
                diag = wp.tile([128, P16, 128], BF16, name="diag", tag="diag")
                nc.vector.tensor_mul(
                    diag[:, 0:8, :], ident_rep[:, 0:8, :],
                    e2[:, 0:8].unsqueeze(2).broadcast_to([128, 8, 128]),
                )
                nc.vector.tensor_mul(
                    diag[:, 8:16, :], ident_rep[:, 8:16, :],
                    e2[:, 8:16].unsqueeze(2).broadcast_to([128, 8, 128]),
                )

                # ---- act += h @ Wh + sum_p w_p * B_p ----
                act, xmms = acts[0]
                acts = acts[1:]
                pe_first = None
                for kk in range(KT):
                    gt = g_lo if kk < 4 else g_hi
                    m = nc.tensor.matmul(
                        out=act[:], lhsT=gt[:, kk % 4, :], rhs=wh_b[:, kk, :],
                        start=False, stop=False,
                    )
                    if pe_first is None:
                        pe_first = m
                for p in range(P16):
                    m = nc.tensor.matmul(
                        out=act[:], lhsT=diag[:, p, :], rhs=B_s[:, p, :],
                        start=False, stop=(p == P16 - 1),
                    )
                if prev_pe_last is not None:
                    add_dep_helper(pe_first.ins, prev_pe_last.ins, sync=False,
                                   reason="pe-queue-order")
                prev_pe_last = m

                # ---- gates, sigmoid-free on doubled cell state ----
                th = wp.tile([128, JS], BF16, name="th", tag="th")
                nc.scalar.activation(
                    out=th[:], in_=act[:, 0:JS],
                    func=mybir.ActivationFunctionType.Tanh, scale=0.5,
                )
                sf = wp.tile([128, HS], BF16, name="sf", tag="sf")
                nc.vector.tensor_scalar(
                    out=sf[:], in0=th[:, HS : 2 * HS], scalar1=0.5, scalar2=0.5,
                    op0=mybir.AluOpType.mult, op1=mybir.AluOpType.add,
                )
                so = wp.tile([128, HS], BF16, name="so", tag="so")
                nc.vector.tensor_scalar(
                    out=so[:], in0=th[:, 2 * HS : 3 * HS], scalar1=0.5, scalar2=0.5,
                    op0=mybir.AluOpType.mult, op1=mybir.AluOpType.add,
                )
                bv = wp.tile([128, HS], F32, name="bv", tag="bv")
                nc.vector.scalar_tensor_tensor(
                    out=bv[:], in0=th[:, 0:HS], scalar=1.0,
                    in1=th[:, 3 * HS : JS],
                    op0=mybir.AluOpType.add, op1=mybir.AluOpType.mult,
                )
                av = wp.tile([128, HS], F32, name="av", tag="av")
                nc.vector.tensor_mul(av[:], sf[:], c2_cur[:])
                c2_new = sp.tile([128, HS], F32, name="c2", tag="c2")
                nc.vector.tensor_add(c2_new[:], av[:], bv[:])
                tcb = wp.tile([128, HS], BF16, name="tcb", tag="tcb")
                nc.scalar.activation(
                    out=tcb[:], in_=c2_new[:],
                    func=mybir.ActivationFunctionType.Tanh, scale=0.5,
                )

                # ---- next payload: h = so*tcb, ship [pscores(h) | hT] ----
                hb = wp.tile([128, HS], BF16, name="hb", tag="hb")
                nc.vector.tensor_mul(hb[:], so[:], tcb[:])
                hTp = tp.tile([128, 128], BF16, name="hTp", tag="tp")
                tr = nc.tensor.transpose(hTp[:], hb[:], ident_b[:])
                if prev_pe_last is not None:
                    add_dep_helper(tr.ins, prev_pe_last.ins, sync=False,
                                   reason="pe-order-tr")
                prev_pe_last = tr
                pay = yp.tile([128, PW], BF16, name="pay", tag="pay")
                nc.scalar.copy(out=pay[:, 2 * P16 :], in_=hTp[:])
                if t + 1 < T:
                    # ship the hT half of the payload as soon as it exists
                    bin_ = dp.tile([128, PW], BF16, name="bin", tag="bin")
                    nc.sync.dma_start(out=bin_[:, 2 * P16 :],
                                      in_=pay[:, 2 * P16 :])
                prod = wp.tile([128, P16, HS], BF16, name="prod", tag="prod")
                nc.vector.tensor_mul(
                    prod[:], afn_b[:],
                    hb[:].unsqueeze(1).broadcast_to([128, P16, HS]),
                )
                pfold = wp.tile([128, P16, HS // 2], BF16, name="pfold", tag="pfold")
                nc.vector.tensor_add(
                    pfold[:], prod[:, :, 0 : HS // 2], prod[:, :, HS // 2 :]
                )
                nc.vector.tensor_reduce(
                    pay[:, 0 : 2 * P16].bitcast(F32), pfold[:],
                    mybir.AxisListType.X, mybir.AluOpType.add,
                )
                if t + 1 < T:
                    nc.sync.dma_start(out=bin_[:, 0 : 2 * P16],
                                      in_=pay[:, 0 : 2 * P16])
                nc.sync.dma_start(out=out_d[t], in_=pay[:, 2 * P16 :])
                c2_cur = c2_new

    nc.compile()
    return nc


def _get_nc():
    global _nc_cache
    if _nc_cache is None:
        _nc_cache = _build()
    return _nc_cache


def _prep_w(W, k, scale_g):
    """(D|H, 4H) -> [128, KT, JS] bf16, g-gate cols doubled."""
    cols = np.concatenate(
        [W[:, g * H + k * HS : g * H + (k + 1) * HS] * (2.0 if (g == 3 and scale_g) else 1.0)
         for g in range(4)], axis=1)
    return np.ascontiguousarray(
        cols.reshape(KT, 128, JS).transpose(1, 0, 2)).astype(BF16_NP)


def _prepare_in_maps(x, A, Wx, Wh, Wattn, b):
    x = np.asarray(x, dtype=np.float32)
    A = np.asarray(A, dtype=np.float32)
    Wx = np.asarray(Wx, dtype=np.float32)
    Wh = np.asarray(Wh, dtype=np.float32)
    Wattn = np.asarray(Wattn, dtype=np.float32)
    b = np.asarray(b, dtype=np.float32)

    xT = np.ascontiguousarray(
        x.transpose(1, 2, 0).reshape(T, KT, 128, N).transpose(0, 2, 1, 3)
    ).astype(BF16_NP)  # (T, 128, KT, N)
    Af = A.reshape(N, H, P16)
    afT = np.ascontiguousarray(
        Af.transpose(1, 2, 0).reshape(KT, 128, P16, N).transpose(1, 0, 2, 3)
    ).astype(BF16_NP)  # (128, KT, P16, N)
    h0 = A.mean(axis=(2, 3))  # (N, H) f32
    ident = np.eye(128, dtype=np.float32).astype(BF16_NP)

    in_maps = []
    for k in range(NC):
        afn = np.ascontiguousarray(
            Af[:, k * HS : (k + 1) * HS, :].transpose(0, 2, 1)
        ).astype(BF16_NP)  # (N, P16, HS)
        bk = np.concatenate(
            [b[g * H + k * HS : g * H + (k + 1) * HS] * (2.0 if g == 3 else 1.0)
             for g in range(4)])
        in_maps.append({
            "xT": xT,
            "wx": _prep_w(Wx, k, True),
            "wh": _prep_w(Wh, k, True),
            "wattn": _prep_w(Wattn, k, True),
            "bvec": bk.reshape(1, JS).astype(BF16_NP),
            "ident": ident,
            "afn": afn,
            "afT": afT,
            "h0": np.ascontiguousarray(h0[:, k * HS : (k + 1) * HS]),
        })
    return in_maps


def _assemble(results):
    # per-core out: (T, HS, N) -> full (N, T, H)
    full = np.empty((N, T, H), dtype=np.float32)
    for k in range(NC):
        full[:, :, k * HS : (k + 1) * HS] = np.asarray(
            results[k]["out"], dtype=np.float32
        ).transpose(2, 0, 1)
    return full


def kernel(**inputs) -> np.ndarray:
    nc = _get_nc()
    in_maps = _prepare_in_maps(**inputs)
    res = run_bass_kernel_spmd(nc, in_maps, core_ids=list(range(NC)))
    return _assemble(res.results)


# revision 17
# speedup vs baseline: 1.0157x; 1.0157x over previous
"""AttentionLSTM Trainium2 kernel, 8-core SPMD, tensor-parallel over 4H.

Core k owns H-slice [128k, 128(k+1)) of each gate (512 act columns
[i_k | f_k | o_k | g_k]) plus the matching c/h state slice.  Per step one
AllGather carries [partial_scores(16 f32) | hT-slice(128)] per batch row
(bf16): gathered hT blocks are the lhsT k-tiles for h@Wh; the 8 partial
score blocks sum to the full attention scores on every core.  The
attention contribution is sum_p w[n,p] * B[n,p,j] with
B[n,p,j] = sum_h Af[n,h,p] Wattn[h,j] precomputed (prologue), applied as
16 diag(w_p) @ B_p matmuls into the same PSUM bank as x@Wx (prefetched
three steps deep so the PE stays busy through the AllGather window) and
h@Wh.

Gate math runs sigmoid-free on a doubled cell state C=2c:
  th = tanh(act/2) (g columns pre-doubled host-side so th_g = tanh(g)),
  sf = 0.5*th_f+0.5, C' = sf*C + (th_i+1)*th_g,  tanh(c') = Tanh(C'*0.5),
  h = (0.5*th_o+0.5) * tanh(c').
h ships as transpose(h) via one PE transpose; partial scores are
sum_hd afn[n,p,hd]*h[n,hd] via mul/fold/reduce on DVE (diag build is
split across DVE and GpSimd).
"""
import numpy as np

from concourse import bacc, tile
from concourse import mybir
from concourse.bass_utils import run_bass_kernel_spmd
from concourse.tile_rust import add_dep_helper

N, T, D, H = 128, 64, 1024, 1024
P16 = 16
NC = 8
HS = H // NC      # 128
JS = 4 * HS       # 512
KT = D // 128     # 8
PW = 2 * P16 + 128  # 160 payload cols: 32 bf16 (16 f32 scores) + 128 hT

F32 = mybir.dt.float32
BF16 = mybir.dt.bfloat16
BF16_NP = mybir.dt.np(mybir.dt.bfloat16)
RG = [list(range(NC))]
PF = 3  # x@Wx prefetch depth

_nc_cache = None


def _build():
    nc = bacc.Bacc("TRN2", target_bir_lowering=False, debug=False, num_devices=NC)

    xT_d = nc.dram_tensor("xT", [T, 128, KT, N], BF16, kind="ExternalInput").ap()
    wx_d = nc.dram_tensor("wx", [128, KT, JS], BF16, kind="ExternalInput").ap()
    wh_d = nc.dram_tensor("wh", [128, KT, JS], BF16, kind="ExternalInput").ap()
    wattn_d = nc.dram_tensor("wattn", [128, KT, JS], BF16, kind="ExternalInput").ap()
    b_d = nc.dram_tensor("bvec", [1, JS], BF16, kind="ExternalInput").ap()
    ident_d = nc.dram_tensor("ident", [128, 128], BF16, kind="ExternalInput").ap()
    afn_d = nc.dram_tensor("afn", [128, P16, HS], BF16, kind="ExternalInput").ap()
    afT_d = nc.dram_tensor("afT", [128, KT, P16, N], BF16, kind="ExternalInput").ap()
    h0_d = nc.dram_tensor("h0", [128, HS], F32, kind="ExternalInput").ap()
    out_d = nc.dram_tensor("out", [T, HS, N], BF16, kind="ExternalOutput").ap()

    with tile.TileContext(nc) as tc:
        with (
            tc.tile_pool(name="const", bufs=1) as cp,
            tc.tile_pool(name="state", bufs=2) as sp,
            tc.tile_pool(name="work", bufs=2) as wp,
            tc.tile_pool(name="xpool", bufs=PF + 1) as xp,
            tc.tile_pool(name="paypool", bufs=2) as yp,
            tc.tile_pool(name="psum", bufs=PF + 1, space="PSUM") as pp,
            tc.tile_pool(name="tpsum", bufs=2, space="PSUM") as tp,
            tc.tile_pool(name="dram", bufs=2, space="DRAM") as dp,
        ):
            # ---------------- constants ----------------
            wx_b = cp.tile([128, KT, JS], BF16, name="wx_b")
            nc.sync.dma_start(out=wx_b[:], in_=wx_d[:])
            wh_b = cp.tile([128, KT, JS], BF16, name="wh_b")
            nc.sync.dma_start(out=wh_b[:], in_=wh_d[:])
            b_b = cp.tile([1, JS], BF16, name="b_b")
            nc.sync.dma_start(out=b_b[:], in_=b_d[:])
            ident_b = cp.tile([128, 128], BF16, name="ident_b")
            nc.sync.dma_start(out=ident_b[:], in_=ident_d[:])
            afn_b = cp.tile([128, P16, HS], BF16, name="afn_b")
            nc.sync.dma_start(out=afn_b[:], in_=afn_d[:])
            ones_b = cp.tile([1, 128], BF16, name="ones_b")
            nc.vector.memset(ones_b[:], 1.0)
            ident_rep = cp.tile([128, P16, 128], BF16, name="ident_rep")
            nc.vector.tensor_copy(
                ident_rep[:], ident_b[:].unsqueeze(1).broadcast_to([128, P16, 128])
            )
            B_s = cp.tile([128, P16, JS], BF16, name="B_s")

            # ---------------- prologue ----------------
            c2_cur = sp.tile([128, HS], F32, name="c2", tag="c2")
            pay = yp.tile([128, PW], BF16, name="pay", tag="pay")
            with (
                tc.tile_pool(name="prol", bufs=1) as pr,
                tc.tile_pool(name="prolp", bufs=2, space="PSUM") as prp,
            ):
                wat_b = pr.tile([128, KT, JS], BF16, name="wat_b")
                nc.sync.dma_start(out=wat_b[:], in_=wattn_d[:])
                afT_b = pr.tile([128, KT, P16, N], BF16, name="afT_b")
                nc.sync.dma_start(out=afT_b[:], in_=afT_d[:])
                h0_t = pr.tile([128, HS], F32, name="h0_t")
                nc.sync.dma_start(out=h0_t[:], in_=h0_d[:])
                nc.vector.tensor_scalar(
                    out=c2_cur[:], in0=h0_t[:], scalar1=2.0, scalar2=None,
                    op0=mybir.AluOpType.mult,
                )
                h0_b = pr.tile([128, HS], BF16, name="h0_b")
                nc.vector.tensor_copy(h0_b[:], h0_t[:])

                # B[n,p,j] = sum_h Af[n,h,p] Wattn[h,j] + b[j]
                # (softmax weights sum to 1, so folding the bias row into
                #  every B_p replaces the per-step bias matmul)
                for p in range(P16):
                    bp = prp.tile([128, JS], F32, name="bp", tag="bp")
                    for kk in range(KT):
                        nc.tensor.matmul(
                            out=bp[:],
                            lhsT=afT_b[:, kk, p, :],
                            rhs=wat_b[:, kk, :],
                            start=(kk == 0), stop=False,
                        )
                    nc.tensor.matmul(
                        out=bp[:], lhsT=ones_b[:], rhs=b_b[:],
                        start=False, stop=True,
                    )
                    nc.scalar.copy(out=B_s[:, p, :], in_=bp[:])

                # payload 0 from h0: [pscores | h0T]
                hTp = tp.tile([128, 128], BF16, name="hTp", tag="tp")
                nc.tensor.transpose(hTp[:], h0_b[:], ident_b[:])
                nc.scalar.copy(out=pay[:, 2 * P16 :], in_=hTp[:])
                prod0 = pr.tile([128, P16, HS], BF16, name="prod0")
                nc.vector.tensor_mul(
                    prod0[:], afn_b[:],
                    h0_b[:].unsqueeze(1).broadcast_to([128, P16, HS]),
                )
                pf0 = pr.tile([128, P16, HS // 2], BF16, name="pf0")
                nc.vector.tensor_add(
                    pf0[:], prod0[:, :, 0 : HS // 2], prod0[:, :, HS // 2 :]
                )
                nc.vector.tensor_reduce(
                    pay[:, 0 : 2 * P16].bitcast(F32), pf0[:],
                    mybir.AxisListType.X, mybir.AluOpType.add,
                )
            bin_ = dp.tile([128, PW], BF16, name="bin", tag="bin")
            nc.sync.dma_start(out=bin_[:], in_=pay[:])

            # ---------------- recurrence ----------------
            def xwx_prefetch(t):
                """x_t @ Wx + b into a fresh act psum bank."""
                xt = xp.tile([128, KT, N], BF16, name="xt", tag="xt")
                nc.sync.dma_start(out=xt[:], in_=xT_d[t])
                act = pp.tile([128, JS], F32, name="act", tag="act")
                mms = []
                for kk in range(KT):
                    mms.append(nc.tensor.matmul(
                        out=act[:],
                        lhsT=xt[:, kk, :],
                        rhs=wx_b[:, kk, :],
                        start=(kk == 0), stop=False,
                    ))
                return act, mms

            acts = [xwx_prefetch(t) for t in range(PF - 1)]
            prev_pe_last = None   # last PE instr queued (order pin)

            for t in range(T):
                # ---- gather payload of h_t (bin_ written last iteration) ----
                bout = dp.tile([NC * 128, PW], BF16, addr_space="Shared",
                               name="bout", tag="bout")
                ag = nc.gpsimd.collective_compute(
                    "AllGather", mybir.AluOpType.bypass, replica_groups=RG,
                    ins=[bin_[:].opt()], outs=[bout[:].opt()],
                )
                # x@Wx prefetch queued to run right at AG completion: its
                # cold matmuls fill the ~4us gather-DMA window and re-warm
                # HAM so the Wh/diag matmuls run at full clock.
                if t + PF - 1 < T:
                    a, mms = xwx_prefetch(t + PF - 1)
                    acts.append((a, mms))
                    add_dep_helper(mms[0].ins, ag.ins, sync=True,
                                   reason="xwx-at-ag-completion")
                    if prev_pe_last is not None:
                        add_dep_helper(mms[0].ins, prev_pe_last.ins,
                                       sync=False, reason="pe-order")
                    prev_pe_last = mms[-1]
                gsrc = bout[:].rearrange("(kk p) f -> p kk f", kk=NC)
                # scores chunk first (small, scalar HWDGE), hT split sync/scalar
                gs = wp.tile([128, NC, 2 * P16], BF16, name="gs", tag="gs")
                nc.scalar.dma_start(out=gs[:], in_=gsrc[:, :, 0 : 2 * P16])
                g_lo = wp.tile([128, 4, 128], BF16, name="g_lo", tag="g_lo")
                nc.sync.dma_start(out=g_lo[:], in_=gsrc[:, 0:4, 2 * P16 :])
                g_hi = wp.tile([128, 4, 128], BF16, name="g_hi", tag="g_hi")
                nc.scalar.dma_start(out=g_hi[:], in_=gsrc[:, 4:8, 2 * P16 :])

                # ---- softmax weights -> diag halves ----
                scores = wp.tile([128, P16], F32, name="scores", tag="scores")
                nc.vector.tensor_reduce(
                    scores[:],
                    gs[:].bitcast(F32).rearrange("n kk q -> n q kk"),
                    mybir.AxisListType.X, mybir.AluOpType.add,
                )
                e_b = wp.tile([128, P16], BF16, name="e_b", tag="e_b")
                den = wp.tile([128, 1], F32, name="den", tag="den")
                nc.scalar.activation(
                    out=e_b[:], in_=scores[:], func=mybir.ActivationFunctionType.Exp,
                    scale=1.0 / 32.0, accum_out=den[:],
                )
                rden = wp.tile([128, 1], F32, name="rden", tag="rden")
                nc.vector.reciprocal(rden[:], den[:])
                diag = wp.tile([128, P16, 128], BF16, name="diag", tag="diag")
                for q in range(4):
                    lo, hi = 4 * q, 4 * q + 4
                    nc.vector.scalar_tensor_tensor(
                        out=diag[:, lo:hi, :],
                        in0=ident_rep[:, lo:hi, :],
                        scalar=rden[:],
                        in1=e_b[:, lo:hi].unsqueeze(2).broadcast_to([128, 4, 128]),
                        op0=mybir.AluOpType.mult,
                        op1=mybir.AluOpType.mult,
                    )

                # ---- act += h @ Wh + sum_p w_p * B_p ----
                act, xmms = acts[0]
                acts = acts[1:]
                pe_first = None
                for kk in range(KT):
                    gt = g_lo if kk < 4 else g_hi
                    m = nc.tensor.matmul(
                        out=act[:], lhsT=gt[:, kk % 4, :], rhs=wh_b[:, kk, :],
                        start=False, stop=False,
                    )
                    if pe_first is None:
                        pe_first = m
                for p in range(P16):
                    m = nc.tensor.matmul(
                        out=act[:], lhsT=diag[:, p, :], rhs=B_s[:, p, :],
                        start=False, stop=(p == P16 - 1),
                    )
                if prev_pe_last is not None:
                    add_dep_helper(pe_first.ins, prev_pe_last.ins, sync=False,
                                   reason="pe-queue-order")
                prev_pe_last = m

                # ---- gates, sigmoid-free on doubled cell state ----
                th = wp.tile([128, JS], BF16, name="th", tag="th")
                nc.scalar.activation(
                    out=th[:], in_=act[:, 0:JS],
                    func=mybir.ActivationFunctionType.Tanh, scale=0.5,
                )
                sf = wp.tile([128, HS], BF16, name="sf", tag="sf")
                nc.vector.tensor_scalar(
                    out=sf[:], in0=th[:, HS : 2 * HS], scalar1=0.5, scalar2=0.5,
                    op0=mybir.AluOpType.mult, op1=mybir.AluOpType.add,
                )
                so = wp.tile([128, HS], BF16, name="so", tag="so")
                nc.vector.tensor_scalar(
                    out=so[:], in0=th[:, 2 * HS : 3 * HS], scalar1=0.5, scalar2=0.5,
                    op0=mybir.AluOpType.mult, op1=mybir.AluOpType.add,
                )
                bv = wp.tile([128, HS], F32, name="bv", tag="bv")
                nc.vector.scalar_tensor_tensor(
                    out=bv[:], in0=th[:, 0:HS], scalar=1.0,
                    in1=th[:, 3 * HS : JS],
                    op0=mybir.AluOpType.add, op1=mybir.AluOpType.mult,
                )
                av = wp.tile([128, HS], F32, name="av", tag="av")
                nc.vector.tensor_mul(av[:], sf[:], c2_cur[:])
                c2_new = sp.tile([128, HS], F32, name="c2", tag="c2")
                nc.vector.tensor_add(c2_new[:], av[:], bv[:])
                tcb = wp.tile([128, HS], BF16, name="tcb", tag="tcb")
                nc.scalar.activation(
                    out=tcb[:], in_=c2_new[:],
                    func=mybir.ActivationFunctionType.Tanh, scale=0.5,
                )

                # ---- next payload: h = so*tcb, ship [pscores(h) | hT] ----
                hb = wp.tile([128, HS], BF16, name="hb", tag="hb")
                nc.vector.tensor_mul(hb[:], so[:], tcb[:])
                hTp = tp.tile([128, 128], BF16, name="hTp", tag="tp")
                tr = nc.tensor.transpose(hTp[:], hb[:], ident_b[:])
                if prev_pe_last is not None:
                    add_dep_helper(tr.ins, prev_pe_last.ins, sync=False,
                                   reason="pe-order-tr")
                prev_pe_last = tr
                pay = yp.tile([128, PW], BF16, name="pay", tag="pay")
                nc.scalar.copy(out=pay[:, 2 * P16 :], in_=hTp[:])
                if t + 1 < T:
                    # ship the hT half of the payload as soon as it exists
                    bin_ = dp.tile([128, PW], BF16, name="bin", tag="bin")
                    nc.sync.dma_start(out=bin_[:, 2 * P16 :],
                                      in_=pay[:, 2 * P16 :])
                prod = wp.tile([128, P16, HS], BF16, name="prod", tag="prod")
                nc.vector.tensor_mul(
                    prod[:], afn_b[:],
                    hb[:].unsqueeze(1).broadcast_to([128, P16, HS]),
                )
                pfold = wp.tile([128, P16, HS // 2], BF16, name="pfold", tag="pfold")
                nc.vector.tensor_add(
                    pfold[:], prod[:, :, 0 : HS // 2], prod[:, :, HS // 2 :]
                )
                nc.vector.tensor_reduce(
                    pay[:, 0 : 2 * P16].bitcast(F32), pfold[:],
                    mybir.AxisListType.X, mybir.AluOpType.add,
                )
                if t + 1 < T:
                    nc.sync.dma_start(out=bin_[:, 0 : 2 * P16],
                                      in_=pay[:, 0 : 2 * P16])
                nc.sync.dma_start(out=out_d[t], in_=pay[:, 2 * P16 :])
                c2_cur = c2_new

    nc.compile()
    return nc


def _get_nc():
    global _nc_cache
    if _nc_cache is None:
        _nc_cache = _build()
    return _nc_cache


def _prep_w(W, k, scale_g):
    """(D|H, 4H) -> [128, KT, JS] bf16, g-gate cols doubled."""
    cols = np.concatenate(
        [W[:, g * H + k * HS : g * H + (k + 1) * HS] * (2.0 if (g == 3 and scale_g) else 1.0)
         for g in range(4)], axis=1)
    return np.ascontiguousarray(
        cols.reshape(KT, 128, JS).transpose(1, 0, 2)).astype(BF16_NP)


def _prepare_in_maps(x, A, Wx, Wh, Wattn, b):
    x = np.asarray(x, dtype=np.float32)
    A = np.asarray(A, dtype=np.float32)
    Wx = np.asarray(Wx, dtype=np.float32)
    Wh = np.asarray(Wh, dtype=np.float32)
    Wattn = np.asarray(Wattn, dtype=np.float32)
    b = np.asarray(b, dtype=np.float32)

    xT = np.ascontiguousarray(
        x.transpose(1, 2, 0).reshape(T, KT, 128, N).transpose(0, 2, 1, 3)
    ).astype(BF16_NP)  # (T, 128, KT, N)
    Af = A.reshape(N, H, P16)
    afT = np.ascontiguousarray(
        Af.transpose(1, 2, 0).reshape(KT, 128, P16, N).transpose(1, 0, 2, 3)
    ).astype(BF16_NP)  # (128, KT, P16, N)
    h0 = A.mean(axis=(2, 3))  # (N, H) f32
    ident = np.eye(128, dtype=np.float32).astype(BF16_NP)

    in_maps = []
    for k in range(NC):
        afn = np.ascontiguousarray(
            Af[:, k * HS : (k + 1) * HS, :].transpose(0, 2, 1)
        ).astype(BF16_NP)  # (N, P16, HS)
        bk = np.concatenate(
            [b[g * H + k * HS : g * H + (k + 1) * HS] * (2.0 if g == 3 else 1.0)
             for g in range(4)])
        in_maps.append({
            "xT": xT,
            "wx": _prep_w(Wx, k, True),
            "wh": _prep_w(Wh, k, True),
            "wattn": _prep_w(Wattn, k, True),
            "bvec": bk.reshape(1, JS).astype(BF16_NP),
            "ident": ident,
            "afn": afn,
            "afT": afT,
            "h0": np.ascontiguousarray(h0[:, k * HS : (k + 1) * HS]),
        })
    return in_maps


def _assemble(results):
    # per-core out: (T, HS, N) -> full (N, T, H)
    full = np.empty((N, T, H), dtype=np.float32)
    for k in range(NC):
        full[:, :, k * HS : (k + 1) * HS] = np.asarray(
            results[k]["out"], dtype=np.float32
        ).transpose(2, 0, 1)
    return full


def kernel(**inputs) -> np.ndarray:
    nc = _get_nc()
    in_maps = _prepare_in_maps(**inputs)
    res = run_bass_kernel_spmd(nc, in_maps, core_ids=list(range(NC)))
    return _assemble(res.results)
